# revision 2
# baseline (speedup 1.0000x reference)
"""Trainium2 Bass kernel v2 for nn_BasicGRUBlock: 2-layer GRU block.

  x = y + z; h1 = GRU0(x); h2 = GRU1(h1); out = y + h2 @ W_lin.T + b_lin

Sharding: data-parallel over batch across 8 cores (8 sequences/core).

v2 design vs v1: the per-step critical chain is
  W@tmp' MMs -> sigmoid(r) -> m=N*S_r -> t=m+gx_n -> tanh -> tmp'=(1-z)*nt
with h' = c + tmp' (c = z*h computed off-chain on Pool) fed to the next
step's recurrent matmuls as TWO moving operands (W@c early + W@tmp' on
the chain), so the h-update itself is off the critical path.  Both
layers' gate math is merged into single wide instructions
([128, layer, chunk, BL] tiles) halving per-engine instruction count and
semaphore traffic.  Group-boundary work (gx0/gx1/final) is spread across
the 16 steps of the body instead of lumped at the end.
"""

import sys

sys.path.insert(0, "/opt/trn_rl_repo")

import numpy as np

import concourse.bass as bass
import concourse.bacc as bacc_mod
import concourse.mybir as mybir
from concourse.bass import ds
from concourse.tile import TileContext

B, T_FULL, I, H, G = 64, 4096, 64, 256, 768
NCORES = 8
BL = B // NCORES  # 8 sequences per core
GRP = 16          # time steps per group
C = GRP * BL      # 128 columns per group (col = k*8 + b)
NJ = 6            # gate chunks of 128 (r: 0-1, z: 2-3, n: 4-5)
NI = 2            # hidden chunks of 128
NL = 2            # layers
F32 = mybir.dt.float32
F32R = mybir.dt.float32r
BF16 = mybir.dt.bfloat16
F8 = mybir.dt.bfloat16  # recurrent-weight storage dtype
WSCL = 1.0    # optional pre-activation scale folded into the weights
WINV = 1.0 / WSCL  # (undone at sigmoid scale= and in the n-path STT ops)

SIG = mybir.ActivationFunctionType.Sigmoid
TANH = mybir.ActivationFunctionType.Tanh
MULT = mybir.AluOpType.mult
ADD = mybir.AluOpType.add
SUB = mybir.AluOpType.subtract


def build_nc(T=T_FULL, unroll_all=False, sig_split=True, spread=True,
             debug=False):
    nc = bacc_mod.Bacc()

    NG = T // GRP
    RPAD_IN = (NG + 2) * C
    RPAD_OUT = (NG + 1) * C

    y_d = nc.declare_dram_parameter("y", [RPAD_IN, I], F32R, isOutput=False)
    z_d = nc.declare_dram_parameter("z", [RPAD_IN, I], F32, isOutput=False)
    whh0_d = nc.declare_dram_parameter("whh0", [128, NJ, NI, 128], F8,
                                       isOutput=False)
    whh1_d = nc.declare_dram_parameter("whh1", [128, NJ, NI, 128], F8,
                                       isOutput=False)
    wih1_d = nc.declare_dram_parameter("wih1", [128, NJ, NI, 128], BF16,
                                       isOutput=False)
    wih0a_d = nc.declare_dram_parameter("wih0a", [I + 1, NJ, 128], BF16,
                                        isOutput=False)
    wlin_d = nc.declare_dram_parameter("wlin", [128, NI, I], BF16,
                                       isOutput=False)
    bias1_d = nc.declare_dram_parameter("bias1", [1, NJ, 128], BF16,
                                        isOutput=False)
    blin_d = nc.declare_dram_parameter("blin", [1, I], BF16, isOutput=False)
    bcall_d = nc.declare_dram_parameter("bcall", [128, NL, NI, BL], BF16,
                                        isOutput=False)
    eyef_d = nc.declare_dram_parameter("eyef", [128, 128], F32,
                                       isOutput=False)
    eyeb_d = nc.declare_dram_parameter("eyeb", [128, 128], BF16,
                                       isOutput=False)
    eyer_d = nc.declare_dram_parameter("eyer", [128, 128], F32R,
                                       isOutput=False)
    out_d = nc.declare_dram_parameter("out", [RPAD_OUT, I], F32,
                                      isOutput=True)
    hdbg_d = None
    if debug:
        hdbg_d = nc.declare_dram_parameter(
            "hdbg", [NG + 1, 128, NL, NI, C], BF16, isOutput=True)

    with TileContext(nc) as tc:
        with (
            tc.tile_pool(name="wpool", bufs=1) as wpool,
            tc.tile_pool(name="gatep", bufs=6) as gatep,
            tc.tile_pool(name="iop", bufs=2) as iop,
            tc.tile_pool(name="ps_a", bufs=2, space="PSUM") as ps_a,
            tc.tile_pool(name="ps_n", bufs=2, space="PSUM") as ps_n,
            tc.tile_pool(name="ps_gx", bufs=2, space="PSUM") as ps_gx,
            tc.tile_pool(name="ps_tp", bufs=1, space="PSUM") as ps_tp,
            tc.tile_pool(name="ps_fin", bufs=1, space="PSUM") as ps_fin,
        ):
            # ---- persistent weights / constants ----
            whh0_t = wpool.tile([128, NJ, NI, 128], F8)
            whh1_t = wpool.tile([128, NJ, NI, 128], F8)
            wih1_t = wpool.tile([128, NJ, NI, 128], BF16)
            wih0a_t = wpool.tile([I + 1, NJ, 128], BF16)
            wlin_t = wpool.tile([128, NI, I], BF16)
            bias1_t = wpool.tile([1, NJ, 128], BF16)
            blin_t = wpool.tile([1, I], BF16)
            bcall_t = wpool.tile([128, NL, NI, BL], BF16)
            bczero_t = wpool.tile([128, NL, NI, BL], BF16)
            eyef_t = wpool.tile([128, 128], F32)
            eyeb_t = wpool.tile([128, 128], BF16)
            eyer_t = wpool.tile([128, 128], F32R)
            ones1_t = wpool.tile([1, 128], BF16)
            onesf_t = wpool.tile([1, 128], F32)

            nc.sync.dma_start(out=whh0_t, in_=whh0_d[:])
            nc.sync.dma_start(out=whh1_t, in_=whh1_d[:])
            nc.sync.dma_start(out=wih1_t, in_=wih1_d[:])
            nc.sync.dma_start(out=wih0a_t, in_=wih0a_d[:])
            nc.sync.dma_start(out=wlin_t, in_=wlin_d[:])
            nc.sync.dma_start(out=bias1_t, in_=bias1_d[:])
            nc.sync.dma_start(out=blin_t, in_=blin_d[:])
            nc.sync.dma_start(out=bcall_t, in_=bcall_d[:])
            nc.sync.dma_start(out=eyef_t, in_=eyef_d[:])
            nc.sync.dma_start(out=eyeb_t, in_=eyeb_d[:])
            nc.sync.dma_start(out=eyer_t, in_=eyer_d[:])
            nc.gpsimd.memset(onesf_t[:], 1.0)
            nc.vector.tensor_copy(ones1_t[:], onesf_t[:])
            # first-body N seed: real b_hh0_n for L0, zeros for L1 (keeps
            # the L1 state identically zero through the warm-up body)
            nc.vector.memset(bczero_t[:].bitcast(BF16), 0.0)
            nc.vector.tensor_copy(bczero_t[:, 0, :, :], bcall_t[:, 0, :, :])

            # ---- parity-pair state tiles ----
            # G_[p]: gx for (L0 group g, L1 group g-1), k-major
            G_ = [wpool.tile([128, GRP, NL, NJ, BL], BF16, name=f"G_{p}")
                  for p in range(2)]
            # H_[p]: [l, i, col]; l=0 -> h1 group g, l=1 -> h2 group g-1
            H_ = [wpool.tile([128, NL, NI, C], BF16, name=f"H_{p}")
                  for p in range(2)]
            # split-matmul carriers: c = z*h, tp = (1-z)*n, k-major
            c_ = [wpool.tile([128, GRP, NL, NI, BL], BF16, name=f"c_{p}")
                  for p in range(2)]
            tp_ = [wpool.tile([128, GRP, NL, NI, BL], BF16, name=f"tp_{p}")
                   for p in range(2)]
            xTa_ = [wpool.tile([I + 1, 128], BF16, name=f"xTa_{p}")
                    for p in range(2)]
            y_ = [wpool.tile([128, I], F32R, name=f"y_{p}") for p in range(2)]
            z_ = [wpool.tile([128, I], F32, name=f"z_{p}") for p in range(2)]

            # init
            nc.vector.tensor_copy(xTa_[0][I: I + 1, :], onesf_t[:])
            nc.vector.tensor_copy(xTa_[1][I: I + 1, :], onesf_t[:])
            nc.vector.memset(H_[1][:], 0.0)
            nc.vector.memset(c_[1][:], 0.0)
            nc.vector.memset(tp_[1][:], 0.0)
            nc.vector.memset(G_[0][:], 0.0)  # L0 slice overwritten below
            nc.vector.memset(y_[1][:].bitcast(F32), 0.0)

            def step(k, par, first):
                """One merged (L0+L1) GRU time step."""
                Gp, Hp = G_[par], H_[par]
                cp, tpp = c_[par], tp_[par]
                if k == 0:
                    csrc, tsrc = c_[1 - par], tp_[1 - par]
                    kprev = GRP - 1
                    Hhp = H_[1 - par]
                    colp = C - BL
                else:
                    csrc, tsrc = cp, tpp
                    kprev = k - 1
                    Hhp = Hp
                    colp = (k - 1) * BL
                A = ps_a.tile([128, NL, 4, BL], F32, tag="A")
                N = ps_n.tile([128, NL, 2, BL], F32, tag="N")
                # folds: seed A with gx(r,z) of both layers in ONE matmul
                # (two start=True matmuls into one bank would clear each
                # other's has_written state), N with b_hh_n
                nc.tensor.matmul(A[:, :, :, :], eyeb_t,
                                 Gp[:, k, :, 0:4, :],
                                 start=True, stop=False,
                                 skip_group_check=True)
                nc.tensor.matmul(N[:, :, :, :], eyeb_t,
                                 bczero_t[:] if first else bcall_t[:],
                                 start=True, stop=False,
                                 skip_group_check=True)
                # W @ c (early) then W @ tmp' (chain) accumulate.  Within
                # the chain block, all A (r,z) writes precede the N writes
                # so the A stop fires (and SIG can start) while the N MMs
                # still stream.
                for src, kk, last in ((csrc, kprev, False),
                                      (tsrc, kprev, True)):
                    for jgrp in ((0, 1), (2, 3), (4, 5)):
                        for l in range(NL):
                            W = whh0_t if l == 0 else whh1_t
                            for j in jgrp:
                                dst = (A[:, l, j, :] if j < 4
                                       else N[:, l, j - 4, :])
                                for i in range(NI):
                                    stop = (last and l == NL - 1
                                            and i == NI - 1
                                            and j in (3, NJ - 1))
                                    nc.tensor.matmul(
                                        dst, W[:, j, i, :],
                                        src[:, kk, l, i, :],
                                        start=False, stop=stop,
                                        skip_group_check=True)
                S = gatep.tile([128, NL, 4, BL], F32, tag="S")
                if sig_split:
                    nc.scalar.activation(S[:, :, 0:2, :], A[:, :, 0:2, :],
                                         SIG, scale=WINV)
                    nc.scalar.activation(S[:, :, 2:4, :], A[:, :, 2:4, :],
                                         SIG, scale=WINV)
                else:
                    nc.scalar.activation(S, A, SIG, scale=WINV)
                # all chain+off ops on DVE/ACT only: buffer-reuse anti-deps
                # then fold into sem floors these engines already track, so
                # every instruction carries at most ONE fresh wait (parkable
                # in the engine wait queue instead of blocking the seq).
                m = gatep.tile([128, NL, 2, BL], F32, tag="m")
                nc.vector.scalar_tensor_tensor(m, N, WINV, S[:, :, 0:2, :],
                                               MULT, MULT)
                t = gatep.tile([128, NL, 2, BL], F32, tag="t")
                nc.vector.scalar_tensor_tensor(t, Gp[:, k, :, 4:6, :], WINV,
                                               m, MULT, ADD)
                zm1 = gatep.tile([128, NL, 2, BL], F32, tag="zm1")
                nc.vector.tensor_scalar(zm1, S[:, :, 2:4, :], -1.0, 1.0,
                                        MULT, ADD)
                nc.vector.tensor_tensor(
                    cp[:, k, :, :, :], S[:, :, 2:4, :],
                    Hhp[:, :, :, colp: colp + BL], MULT)
                nt = gatep.tile([128, NL, 2, BL], F32, tag="nt")
                nc.scalar.activation(nt, t, TANH)
                nc.vector.tensor_tensor(tpp[:, k, :, :, :], nt, zm1, MULT)
                nc.vector.tensor_tensor(
                    Hp[:, :, :, k * BL: (k + 1) * BL],
                    cp[:, k, :, :, :], tpp[:, k, :, :, :], ADD)

            def gx_copy(j, dst, gps):
                if j % 2 == 0:
                    nc.vector.tensor_copy(dst, gps)
                else:
                    nc.scalar.copy(dst, gps)

            def gx1_piece(par, j, c0, c1):
                """W_ih1 @ H1(g) cols [c0,c1) for gate chunk j ->
                G_[1-par] L1 slice."""
                w = c1 - c0
                gps = ps_gx.tile([128, w], F32, tag="gx")
                for i in range(NI):
                    nc.tensor.matmul(gps, wih1_t[:, j, i, :],
                                     H_[par][:, 0, i, c0:c1],
                                     start=(i == 0), stop=False,
                                     skip_group_check=True)
                nc.tensor.matmul(gps, bias1_t[:, j, :], ones1_t[:, 0:w],
                                 start=False, stop=True,
                                 skip_group_check=True)
                k0, k1 = c0 // BL, c1 // BL
                gx_copy(j, G_[1 - par][:, k0:k1, 1, j, :], gps)

            def gx0_piece(par, j):
                """W_ih0a @ xTa(g+1) for gate chunk j -> G_[1-par] L0."""
                gps = ps_gx.tile([128, C], F32, tag="gx")
                nc.tensor.matmul(gps, wih0a_t[:, j, :], xTa_[1 - par],
                                 start=True, stop=True)
                gx_copy(j, G_[1 - par][:, :, 0, j, :], gps)

            def final_head(par):
                """Seed fp with y(g-1) + b_lin at body start, before the
                y buffer is overwritten by the next group's prefetch."""
                fp = ps_fin.tile([128, I], F32, tag="fin")
                nc.tensor.matmul(fp, eyer_t, y_[1 - par],
                                 start=True, stop=False)
                nc.tensor.matmul(fp, ones1_t, blin_t,
                                 start=False, stop=False)
                return fp

            def final_tail(fp, r_o, par):
                """fp += W_lin @ H2(g-1) (complete at body end); store."""
                nc.tensor.matmul(fp, H_[par][:, 1, 0, :], wlin_t[:, 0, :],
                                 start=False, stop=False)
                nc.tensor.matmul(fp, H_[par][:, 1, 1, :], wlin_t[:, 1, :],
                                 start=False, stop=True)
                o_t = iop.tile([128, I], F32, tag="o")
                nc.scalar.copy(o_t, fp)
                nc.sync.dma_start(out=out_d[ds(r_o, C), :], in_=o_t)

            def bulk0_head(r_y, par):
                nc.sync.dma_start(out=y_[par], in_=y_d[ds(r_y, C), :])
                nc.sync.dma_start(out=z_[par], in_=z_d[ds(r_y, C), :])

            def bulk0_mid(par):
                x_t = iop.tile([128, I], F32, tag="x")
                nc.gpsimd.tensor_tensor(x_t, y_[par].bitcast(F32), z_[par],
                                        ADD)
                tp = ps_tp.tile([I, 128], F32, tag="tp")
                nc.tensor.transpose(tp, x_t, eyef_t)
                nc.scalar.copy(xTa_[par][0:I, :], tp)

            def body(r0, par, first=False):
                fp = None
                for k in range(GRP):
                    step(k, par, first)
                    if not spread:
                        continue
                    if k == 0:
                        # must precede bulk0_head: reads y of group g-1
                        fp = final_head(par)
                    elif k == 1:
                        # prefetch next group's inputs into parity 1-par
                        bulk0_head(r0 + C, 1 - par)
                    elif k == 8:
                        bulk0_mid(1 - par)
                    elif 9 <= k <= 14:
                        gx0_piece(par, k - 9)
                        if k >= 10:
                            gx1_piece(par, k - 10, 0, 64)
                if not spread:
                    fp = final_head(par)
                    for j in range(NJ):
                        gx1_piece(par, j, 0, C)
                    final_tail(fp, r0, par)
                    bulk0_head(r0 + C, 1 - par)
                    bulk0_mid(1 - par)
                    for j in range(NJ):
                        gx0_piece(par, j)
                else:
                    gx1_piece(par, 5, 0, 64)
                    final_tail(fp, r0, par)
                    gx1_piece(par, 4, 64, C)
                    gx1_piece(par, 5, 64, C)
                    for j in range(4):
                        gx1_piece(par, j, 64, C)
                if debug:
                    nc.sync.dma_start(out=hdbg_d[r0 // C], in_=H_[par][:])

            # prologue: group 0 L0 inputs -> parity 0
            bulk0_head(0, 0)
            bulk0_mid(0)
            for j in range(NJ):
                gx0_piece(1, j)  # writes G_[0] L0 slice

            if unroll_all:
                for g in range(NG + 1):
                    body(g * C, g % 2, first=(g == 0))
            else:
                body(0, 0, first=True)
                with tc.For_i(C, (NG + 1) * C, 2 * C,
                              staggered_reset=True) as iv:
                    body(iv, 1)
                    body(iv + C, 0)

    nc.compile()
    return nc


def prep_weights(W_ih0, W_hh0, b_ih0, b_hh0, W_ih1, W_hh1, b_ih1, b_hh1,
                 W_lin, b_lin):
    """Host-side weight folding into gate-major bf16 layouts."""
    import ml_dtypes
    bf = ml_dtypes.bfloat16
    f8 = ml_dtypes.bfloat16
    f = np.float32
    s = f(WSCL)

    def _lay(W):  # [768, 256] -> [128, 6, 2, 128]
        return np.ascontiguousarray(
            W.reshape(NJ, 128, NI, 128).transpose(3, 0, 2, 1))

    # recurrent weights: fp8e3m4 at x WSCL; every other contribution to the
    # gate pre-activations (gx weights, biases) carries the same x WSCL so
    # the PSUM accumulation is uniformly scaled, undone at sigmoid (scale=)
    # and in the n-path STT ops.
    bias0 = (b_ih0 + np.concatenate([b_hh0[: 2 * H], np.zeros(H, f)])).astype(f)
    bias1 = (b_ih1 + np.concatenate([b_hh1[: 2 * H], np.zeros(H, f)])).astype(f)

    wih0a = np.zeros((I + 1, NJ, 128), f)
    wih0a[:I] = W_ih0.reshape(NJ, 128, I).transpose(2, 0, 1)
    wih0a[I] = bias0.reshape(NJ, 128)

    # bcall: [128, layer, i, BL] = b_hh_l n-part broadcast over batch
    bcall = np.zeros((128, NL, NI, BL), f)
    for l, bh in enumerate((b_hh0, b_hh1)):
        bcall[:, l] = np.broadcast_to(
            bh[2 * H:].reshape(NI, 128).T[:, :, None], (128, NI, BL))

    return {
        "whh0": (_lay(W_hh0) * s).astype(f8),
        "whh1": (_lay(W_hh1) * s).astype(f8),
        "wih1": (_lay(W_ih1) * s).astype(bf),
        "wih0a": (wih0a * s).astype(bf),
        "wlin": np.ascontiguousarray(
            W_lin.T.reshape(NI, 128, I).transpose(1, 0, 2)).astype(bf),
        "bias1": (bias1.reshape(1, NJ, 128) * s).astype(bf),
        "blin": b_lin.reshape(1, I).astype(bf),
        "bcall": np.ascontiguousarray(bcall * s).astype(bf),
        "eyef": np.eye(128, dtype=f),
        "eyeb": np.eye(128, dtype=bf),
        "eyer": np.eye(128, dtype=f),
    }


def prep_seq(a, T):
    """[BLc, T, I] f32 -> padded [RPAD_IN, I] rows (t*BLc+b order)."""
    BLc = a.shape[0]
    NG = T // GRP
    r = np.ascontiguousarray(a.transpose(1, 0, 2)).reshape(T * BLc, I)
    pad = np.zeros(((NG + 2) * GRP * BLc - T * BLc, I), np.float32)
    return np.concatenate([r, pad], axis=0)


def unprep_out(o, T):
    """[RPAD_OUT, I] -> [BL, T, I] (drop first pad group)."""
    o = o[C:].reshape(T, BL, I)
    return np.ascontiguousarray(o.transpose(1, 0, 2))


_NC_CACHE = {}


def kernel(z, y, W_ih0, W_hh0, b_ih0, b_hh0, W_ih1, W_hh1, b_ih1, b_hh1,
           W_lin, b_lin, _trace=False):
    """Full-input entry point: shards over 8 cores, returns full output."""
    from concourse.bass_utils import run_bass_kernel_spmd

    z = np.asarray(z, np.float32)
    y = np.asarray(y, np.float32)
    T = z.shape[1]
    if T not in _NC_CACHE:
        _NC_CACHE[T] = build_nc(T=T)
    nc = _NC_CACHE[T]

    wmaps = prep_weights(
        np.asarray(W_ih0), np.asarray(W_hh0), np.asarray(b_ih0),
        np.asarray(b_hh0), np.asarray(W_ih1), np.asarray(W_hh1),
        np.asarray(b_ih1), np.asarray(b_hh1), np.asarray(W_lin),
        np.asarray(b_lin))

    in_maps = []
    for cid in range(NCORES):
        sl = slice(cid * BL, (cid + 1) * BL)
        m = {"y": prep_seq(y[sl], T), "z": prep_seq(z[sl], T)}
        m.update(wmaps)
        in_maps.append(m)

    res = run_bass_kernel_spmd(nc, in_maps, list(range(NCORES)),
                               trace=_trace)
    outs = [unprep_out(res.results[cid]["out"], T) for cid in range(NCORES)]
    full = np.concatenate(outs, axis=0).astype(np.float32)
    if _trace:
        return full, res
    return full


# revision 3
# speedup vs baseline: 1.0428x; 1.0428x over previous
"""Trainium2 Bass kernel v2 for nn_BasicGRUBlock: 2-layer GRU block.

  x = y + z; h1 = GRU0(x); h2 = GRU1(h1); out = y + h2 @ W_lin.T + b_lin

Sharding: data-parallel over batch across 8 cores (8 sequences/core).

v2 design vs v1: the per-step critical chain is
  W@tmp' MMs -> sigmoid(r) -> m=N*S_r -> t=m+gx_n -> tanh -> tmp'=(1-z)*nt
with h' = c + tmp' (c = z*h computed off-chain on Pool) fed to the next
step's recurrent matmuls as TWO moving operands (W@c early + W@tmp' on
the chain), so the h-update itself is off the critical path.  Both
layers' gate math is merged into single wide instructions
([128, layer, chunk, BL] tiles) halving per-engine instruction count and
semaphore traffic.  Group-boundary work (gx0/gx1/final) is spread across
the 16 steps of the body instead of lumped at the end.
"""

import sys

sys.path.insert(0, "/opt/trn_rl_repo")

import numpy as np

import concourse.bass as bass
import concourse.bacc as bacc_mod
import concourse.mybir as mybir
from concourse.bass import ds
from concourse.tile import TileContext

B, T_FULL, I, H, G = 64, 4096, 64, 256, 768
NCORES = 8
BL = B // NCORES  # 8 sequences per core
GRP = 16          # time steps per group
C = GRP * BL      # 128 columns per group (col = k*8 + b)
NJ = 6            # gate chunks of 128 (r: 0-1, z: 2-3, n: 4-5)
NI = 2            # hidden chunks of 128
NL = 2            # layers
F32 = mybir.dt.float32
F32R = mybir.dt.float32r
BF16 = mybir.dt.bfloat16
F8 = mybir.dt.bfloat16  # recurrent-weight storage dtype
WSCL = 1.0    # optional pre-activation scale folded into the weights
WINV = 1.0 / WSCL  # (undone at sigmoid scale= and in the n-path STT ops)

SIG = mybir.ActivationFunctionType.Sigmoid
TANH = mybir.ActivationFunctionType.Tanh
MULT = mybir.AluOpType.mult
ADD = mybir.AluOpType.add
SUB = mybir.AluOpType.subtract


def build_nc(T=T_FULL, unroll_all=False, sig_split=True, spread=True,
             debug=False):
    nc = bacc_mod.Bacc()

    NG = T // GRP
    RPAD_IN = (NG + 2) * C
    RPAD_OUT = (NG + 1) * C

    y_d = nc.declare_dram_parameter("y", [RPAD_IN, I], F32R, isOutput=False)
    z_d = nc.declare_dram_parameter("z", [RPAD_IN, I], F32, isOutput=False)
    whh0_d = nc.declare_dram_parameter("whh0", [128, NJ, NI, 128], F8,
                                       isOutput=False)
    whh1_d = nc.declare_dram_parameter("whh1", [128, NJ, NI, 128], F8,
                                       isOutput=False)
    wih1_d = nc.declare_dram_parameter("wih1", [128, NJ, NI, 128], BF16,
                                       isOutput=False)
    wih0a_d = nc.declare_dram_parameter("wih0a", [I + 1, NJ, 128], BF16,
                                        isOutput=False)
    wlin_d = nc.declare_dram_parameter("wlin", [128, NI, I], BF16,
                                       isOutput=False)
    bias1_d = nc.declare_dram_parameter("bias1", [1, NJ, 128], BF16,
                                        isOutput=False)
    blin_d = nc.declare_dram_parameter("blin", [1, I], BF16, isOutput=False)
    bcall_d = nc.declare_dram_parameter("bcall", [128, NL, NI, BL], BF16,
                                        isOutput=False)
    eyef_d = nc.declare_dram_parameter("eyef", [128, 128], F32,
                                       isOutput=False)
    eyeb_d = nc.declare_dram_parameter("eyeb", [128, 128], BF16,
                                       isOutput=False)
    eyer_d = nc.declare_dram_parameter("eyer", [128, 128], F32R,
                                       isOutput=False)
    out_d = nc.declare_dram_parameter("out", [RPAD_OUT, I], F32,
                                      isOutput=True)
    hdbg_d = None
    if debug:
        hdbg_d = nc.declare_dram_parameter(
            "hdbg", [NG + 1, 128, NL, NI, C], BF16, isOutput=True)

    with TileContext(nc) as tc:
        with (
            tc.tile_pool(name="wpool", bufs=1) as wpool,
            tc.tile_pool(name="gatep", bufs=6) as gatep,
            tc.tile_pool(name="iop", bufs=2) as iop,
            tc.tile_pool(name="ps_a", bufs=2, space="PSUM") as ps_a,
            tc.tile_pool(name="ps_n", bufs=2, space="PSUM") as ps_n,
            tc.tile_pool(name="ps_gx", bufs=2, space="PSUM") as ps_gx,
            tc.tile_pool(name="ps_tp", bufs=1, space="PSUM") as ps_tp,
            tc.tile_pool(name="ps_fin", bufs=1, space="PSUM") as ps_fin,
        ):
            # ---- persistent weights / constants ----
            whh0_t = wpool.tile([128, NJ, NI, 128], F8)
            whh1_t = wpool.tile([128, NJ, NI, 128], F8)
            wih1_t = wpool.tile([128, NJ, NI, 128], BF16)
            wih0a_t = wpool.tile([I + 1, NJ, 128], BF16)
            wlin_t = wpool.tile([128, NI, I], BF16)
            bias1_t = wpool.tile([1, NJ, 128], BF16)
            blin_t = wpool.tile([1, I], BF16)
            bcall_t = wpool.tile([128, NL, NI, BL], BF16)
            bczero_t = wpool.tile([128, NL, NI, BL], BF16)
            eyef_t = wpool.tile([128, 128], F32)
            eyeb_t = wpool.tile([128, 128], BF16)
            eyer_t = wpool.tile([128, 128], F32R)
            ones1_t = wpool.tile([1, 128], BF16)
            onesf_t = wpool.tile([1, 128], F32)

            nc.sync.dma_start(out=whh0_t, in_=whh0_d[:])
            nc.sync.dma_start(out=whh1_t, in_=whh1_d[:])
            nc.sync.dma_start(out=wih1_t, in_=wih1_d[:])
            nc.sync.dma_start(out=wih0a_t, in_=wih0a_d[:])
            nc.sync.dma_start(out=wlin_t, in_=wlin_d[:])
            nc.sync.dma_start(out=bias1_t, in_=bias1_d[:])
            nc.sync.dma_start(out=blin_t, in_=blin_d[:])
            nc.sync.dma_start(out=bcall_t, in_=bcall_d[:])
            nc.sync.dma_start(out=eyef_t, in_=eyef_d[:])
            nc.sync.dma_start(out=eyeb_t, in_=eyeb_d[:])
            nc.sync.dma_start(out=eyer_t, in_=eyer_d[:])
            nc.gpsimd.memset(onesf_t[:], 1.0)
            nc.vector.tensor_copy(ones1_t[:], onesf_t[:])
            # first-body N seed: real b_hh0_n for L0, zeros for L1 (keeps
            # the L1 state identically zero through the warm-up body)
            nc.vector.memset(bczero_t[:].bitcast(BF16), 0.0)
            nc.vector.tensor_copy(bczero_t[:, 0, :, :], bcall_t[:, 0, :, :])

            # ---- parity-pair state tiles ----
            # G_[p]: gx for (L0 group g, L1 group g-1), k-major
            G_ = [wpool.tile([128, GRP, NL, NJ, BL], BF16, name=f"G_{p}")
                  for p in range(2)]
            # H_[p]: [l, i, col]; l=0 -> h1 group g, l=1 -> h2 group g-1
            H_ = [wpool.tile([128, NL, NI, C], BF16, name=f"H_{p}")
                  for p in range(2)]
            # split-matmul carriers: c = z*h, tp = (1-z)*n, k-major
            c_ = [wpool.tile([128, GRP, NL, NI, BL], BF16, name=f"c_{p}")
                  for p in range(2)]
            tp_ = [wpool.tile([128, GRP, NL, NI, BL], BF16, name=f"tp_{p}")
                   for p in range(2)]
            xTa_ = [wpool.tile([I + 1, 128], BF16, name=f"xTa_{p}")
                    for p in range(2)]
            y_ = [wpool.tile([128, I], F32R, name=f"y_{p}") for p in range(2)]
            z_ = [wpool.tile([128, I], F32, name=f"z_{p}") for p in range(2)]

            # init
            nc.vector.tensor_copy(xTa_[0][I: I + 1, :], onesf_t[:])
            nc.vector.tensor_copy(xTa_[1][I: I + 1, :], onesf_t[:])
            nc.vector.memset(H_[1][:], 0.0)
            nc.vector.memset(c_[1][:], 0.0)
            nc.vector.memset(tp_[1][:], 0.0)
            nc.vector.memset(G_[0][:], 0.0)  # L0 slice overwritten below
            nc.vector.memset(y_[1][:].bitcast(F32), 0.0)

            def step(k, par, first):
                """One merged (L0+L1) GRU time step."""
                Gp, Hp = G_[par], H_[par]
                cp, tpp = c_[par], tp_[par]
                if k == 0:
                    csrc, tsrc = c_[1 - par], tp_[1 - par]
                    kprev = GRP - 1
                    Hhp = H_[1 - par]
                    colp = C - BL
                else:
                    csrc, tsrc = cp, tpp
                    kprev = k - 1
                    Hhp = Hp
                    colp = (k - 1) * BL
                A = ps_a.tile([128, NL, 4, BL], F32, tag="A")
                N = ps_n.tile([128, NL, 2, BL], F32, tag="N")
                # folds: seed A with gx(r,z) of both layers in ONE matmul
                # (two start=True matmuls into one bank would clear each
                # other's has_written state), N with b_hh_n
                nc.tensor.matmul(A[:, :, :, :], eyeb_t,
                                 Gp[:, k, :, 0:4, :],
                                 start=True, stop=False,
                                 skip_group_check=True)
                nc.tensor.matmul(N[:, :, :, :], eyeb_t,
                                 bczero_t[:] if first else bcall_t[:],
                                 start=True, stop=False,
                                 skip_group_check=True)
                # single W @ h block (r, z, n order: SIG_r's region
                # completes first, the n writes (m's input) stream last)
                for jgrp in ((0, 1), (2, 3), (4, 5)):
                    for l in range(NL):
                        W = whh0_t if l == 0 else whh1_t
                        for j in jgrp:
                            dst = (A[:, l, j, :] if j < 4
                                   else N[:, l, j - 4, :])
                            for i in range(NI):
                                stop = (l == NL - 1 and i == NI - 1
                                        and j in (3, NJ - 1))
                                nc.tensor.matmul(
                                    dst, W[:, j, i, :],
                                    Hhp[:, l, i, colp: colp + BL],
                                    start=False, stop=stop,
                                    skip_group_check=True)
                S = gatep.tile([128, NL, 4, BL], F32, tag="S")
                if sig_split:
                    nc.scalar.activation(S[:, :, 0:2, :], A[:, :, 0:2, :],
                                         SIG, scale=WINV)
                    nc.scalar.activation(S[:, :, 2:4, :], A[:, :, 2:4, :],
                                         SIG, scale=WINV)
                else:
                    nc.scalar.activation(S, A, SIG, scale=WINV)
                # all chain+off ops on DVE/ACT only: buffer-reuse anti-deps
                # then fold into sem floors these engines already track, so
                # every instruction carries at most ONE fresh wait (parkable
                # in the engine wait queue instead of blocking the seq).
                m = gatep.tile([128, NL, 2, BL], F32, tag="m")
                nc.vector.scalar_tensor_tensor(m, N, WINV, S[:, :, 0:2, :],
                                               MULT, MULT)
                t = gatep.tile([128, NL, 2, BL], F32, tag="t")
                nc.vector.scalar_tensor_tensor(t, Gp[:, k, :, 4:6, :], WINV,
                                               m, MULT, ADD)
                zm1 = gatep.tile([128, NL, 2, BL], F32, tag="zm1")
                nc.vector.tensor_scalar(zm1, S[:, :, 2:4, :], -1.0, 1.0,
                                        MULT, ADD)
                nc.vector.tensor_tensor(
                    cp[:, k, :, :, :], S[:, :, 2:4, :],
                    Hhp[:, :, :, colp: colp + BL], MULT)
                nt = gatep.tile([128, NL, 2, BL], F32, tag="nt")
                nc.scalar.activation(nt, t, TANH)
                nc.vector.tensor_tensor(tpp[:, k, :, :, :], nt, zm1, MULT)
                nc.vector.tensor_tensor(
                    Hp[:, :, :, k * BL: (k + 1) * BL],
                    cp[:, k, :, :, :], tpp[:, k, :, :, :], ADD)

            def gx_copy(j, dst, gps):
                if j % 2 == 0:
                    nc.vector.tensor_copy(dst, gps)
                else:
                    nc.scalar.copy(dst, gps)

            def gx1_piece(par, j, c0, c1):
                """W_ih1 @ H1(g) cols [c0,c1) for gate chunk j ->
                G_[1-par] L1 slice."""
                w = c1 - c0
                gps = ps_gx.tile([128, w], F32, tag="gx")
                for i in range(NI):
                    nc.tensor.matmul(gps, wih1_t[:, j, i, :],
                                     H_[par][:, 0, i, c0:c1],
                                     start=(i == 0), stop=False,
                                     skip_group_check=True)
                nc.tensor.matmul(gps, bias1_t[:, j, :], ones1_t[:, 0:w],
                                 start=False, stop=True,
                                 skip_group_check=True)
                k0, k1 = c0 // BL, c1 // BL
                gx_copy(j, G_[1 - par][:, k0:k1, 1, j, :], gps)

            def gx0_piece(par, j):
                """W_ih0a @ xTa(g+1) for gate chunk j -> G_[1-par] L0."""
                gps = ps_gx.tile([128, C], F32, tag="gx")
                nc.tensor.matmul(gps, wih0a_t[:, j, :], xTa_[1 - par],
                                 start=True, stop=True)
                gx_copy(j, G_[1 - par][:, :, 0, j, :], gps)

            def final_head(par):
                """Seed fp with y(g-1) + b_lin at body start, before the
                y buffer is overwritten by the next group's prefetch."""
                fp = ps_fin.tile([128, I], F32, tag="fin")
                nc.tensor.matmul(fp, eyer_t, y_[1 - par],
                                 start=True, stop=False)
                nc.tensor.matmul(fp, ones1_t, blin_t,
                                 start=False, stop=False)
                return fp

            def final_tail(fp, r_o, par):
                """fp += W_lin @ H2(g-1) (complete at body end); store."""
                nc.tensor.matmul(fp, H_[par][:, 1, 0, :], wlin_t[:, 0, :],
                                 start=False, stop=False)
                nc.tensor.matmul(fp, H_[par][:, 1, 1, :], wlin_t[:, 1, :],
                                 start=False, stop=True)
                o_t = iop.tile([128, I], F32, tag="o")
                nc.scalar.copy(o_t, fp)
                nc.sync.dma_start(out=out_d[ds(r_o, C), :], in_=o_t)

            def bulk0_head(r_y, par):
                nc.sync.dma_start(out=y_[par], in_=y_d[ds(r_y, C), :])
                nc.sync.dma_start(out=z_[par], in_=z_d[ds(r_y, C), :])

            def bulk0_mid(par):
                x_t = iop.tile([128, I], F32, tag="x")
                nc.gpsimd.tensor_tensor(x_t, y_[par].bitcast(F32), z_[par],
                                        ADD)
                tp = ps_tp.tile([I, 128], F32, tag="tp")
                nc.tensor.transpose(tp, x_t, eyef_t)
                nc.scalar.copy(xTa_[par][0:I, :], tp)

            def body(r0, par, first=False):
                fp = None
                for k in range(GRP):
                    step(k, par, first)
                    if not spread:
                        continue
                    if k == 0:
                        # must precede bulk0_head: reads y of group g-1
                        fp = final_head(par)
                    elif k == 1:
                        # prefetch next group's inputs into parity 1-par
                        bulk0_head(r0 + C, 1 - par)
                    elif k == 8:
                        bulk0_mid(1 - par)
                    elif 9 <= k <= 14:
                        gx0_piece(par, k - 9)
                        if k >= 10:
                            gx1_piece(par, k - 10, 0, 64)
                if not spread:
                    fp = final_head(par)
                    for j in range(NJ):
                        gx1_piece(par, j, 0, C)
                    final_tail(fp, r0, par)
                    bulk0_head(r0 + C, 1 - par)
                    bulk0_mid(1 - par)
                    for j in range(NJ):
                        gx0_piece(par, j)
                else:
                    gx1_piece(par, 5, 0, 64)
                    final_tail(fp, r0, par)
                    gx1_piece(par, 4, 64, C)
                    gx1_piece(par, 5, 64, C)
                    for j in range(4):
                        gx1_piece(par, j, 64, C)
                if debug:
                    nc.sync.dma_start(out=hdbg_d[r0 // C], in_=H_[par][:])

            # prologue: group 0 L0 inputs -> parity 0
            bulk0_head(0, 0)
            bulk0_mid(0)
            for j in range(NJ):
                gx0_piece(1, j)  # writes G_[0] L0 slice

            if unroll_all:
                for g in range(NG + 1):
                    body(g * C, g % 2, first=(g == 0))
            else:
                body(0, 0, first=True)
                with tc.For_i(C, (NG + 1) * C, 2 * C,
                              staggered_reset=True) as iv:
                    body(iv, 1)
                    body(iv + C, 0)

    nc.compile()
    return nc


def prep_weights(W_ih0, W_hh0, b_ih0, b_hh0, W_ih1, W_hh1, b_ih1, b_hh1,
                 W_lin, b_lin):
    """Host-side weight folding into gate-major bf16 layouts."""
    import ml_dtypes
    bf = ml_dtypes.bfloat16
    f8 = ml_dtypes.bfloat16
    f = np.float32
    s = f(WSCL)

    def _lay(W):  # [768, 256] -> [128, 6, 2, 128]
        return np.ascontiguousarray(
            W.reshape(NJ, 128, NI, 128).transpose(3, 0, 2, 1))

    # recurrent weights: fp8e3m4 at x WSCL; every other contribution to the
    # gate pre-activations (gx weights, biases) carries the same x WSCL so
    # the PSUM accumulation is uniformly scaled, undone at sigmoid (scale=)
    # and in the n-path STT ops.
    bias0 = (b_ih0 + np.concatenate([b_hh0[: 2 * H], np.zeros(H, f)])).astype(f)
    bias1 = (b_ih1 + np.concatenate([b_hh1[: 2 * H], np.zeros(H, f)])).astype(f)

    wih0a = np.zeros((I + 1, NJ, 128), f)
    wih0a[:I] = W_ih0.reshape(NJ, 128, I).transpose(2, 0, 1)
    wih0a[I] = bias0.reshape(NJ, 128)

    # bcall: [128, layer, i, BL] = b_hh_l n-part broadcast over batch
    bcall = np.zeros((128, NL, NI, BL), f)
    for l, bh in enumerate((b_hh0, b_hh1)):
        bcall[:, l] = np.broadcast_to(
            bh[2 * H:].reshape(NI, 128).T[:, :, None], (128, NI, BL))

    return {
        "whh0": (_lay(W_hh0) * s).astype(f8),
        "whh1": (_lay(W_hh1) * s).astype(f8),
        "wih1": (_lay(W_ih1) * s).astype(bf),
        "wih0a": (wih0a * s).astype(bf),
        "wlin": np.ascontiguousarray(
            W_lin.T.reshape(NI, 128, I).transpose(1, 0, 2)).astype(bf),
        "bias1": (bias1.reshape(1, NJ, 128) * s).astype(bf),
        "blin": b_lin.reshape(1, I).astype(bf),
        "bcall": np.ascontiguousarray(bcall * s).astype(bf),
        "eyef": np.eye(128, dtype=f),
        "eyeb": np.eye(128, dtype=bf),
        "eyer": np.eye(128, dtype=f),
    }


def prep_seq(a, T):
    """[BLc, T, I] f32 -> padded [RPAD_IN, I] rows (t*BLc+b order)."""
    BLc = a.shape[0]
    NG = T // GRP
    r = np.ascontiguousarray(a.transpose(1, 0, 2)).reshape(T * BLc, I)
    pad = np.zeros(((NG + 2) * GRP * BLc - T * BLc, I), np.float32)
    return np.concatenate([r, pad], axis=0)


def unprep_out(o, T):
    """[RPAD_OUT, I] -> [BL, T, I] (drop first pad group)."""
    o = o[C:].reshape(T, BL, I)
    return np.ascontiguousarray(o.transpose(1, 0, 2))


_NC_CACHE = {}


def kernel(z, y, W_ih0, W_hh0, b_ih0, b_hh0, W_ih1, W_hh1, b_ih1, b_hh1,
           W_lin, b_lin, _trace=False):
    """Full-input entry point: shards over 8 cores, returns full output."""
    from concourse.bass_utils import run_bass_kernel_spmd

    z = np.asarray(z, np.float32)
    y = np.asarray(y, np.float32)
    T = z.shape[1]
    if T not in _NC_CACHE:
        _NC_CACHE[T] = build_nc(T=T)
    nc = _NC_CACHE[T]

    wmaps = prep_weights(
        np.asarray(W_ih0), np.asarray(W_hh0), np.asarray(b_ih0),
        np.asarray(b_hh0), np.asarray(W_ih1), np.asarray(W_hh1),
        np.asarray(b_ih1), np.asarray(b_hh1), np.asarray(W_lin),
        np.asarray(b_lin))

    in_maps = []
    for cid in range(NCORES):
        sl = slice(cid * BL, (cid + 1) * BL)
        m = {"y": prep_seq(y[sl], T), "z": prep_seq(z[sl], T)}
        m.update(wmaps)
        in_maps.append(m)

    res = run_bass_kernel_spmd(nc, in_maps, list(range(NCORES)),
                               trace=_trace)
    outs = [unprep_out(res.results[cid]["out"], T) for cid in range(NCORES)]
    full = np.concatenate(outs, axis=0).astype(np.float32)
    if _trace:
        return full, res
    return full


# revision 5
# speedup vs baseline: 1.1796x; 1.1311x over previous
"""Trainium2 Bass kernel v2 for nn_BasicGRUBlock: 2-layer GRU block.

  x = y + z; h1 = GRU0(x); h2 = GRU1(h1); out = y + h2 @ W_lin.T + b_lin

Sharding: data-parallel over batch across 8 cores (8 sequences/core).

Design: per-step critical chain is
  W@h MMs (r,z,n order) -> sigmoid(r) -> m=N*S_r -> t=m+gx_n -> tanh
  -> tmp'=(1-z)*nt -> h'=c+tmp'
Both layers' gate math is merged into single wide instructions
([128, layer, chunk, BL] tiles), halving per-engine instruction count
and semaphore traffic; every chain instruction carries at most one
fresh semaphore wait (all chain-tile consumers stay on DVE/ACT so
buffer-reuse anti-deps fold into tracked sem floors).  Group-boundary
work (gx0/gx1/final/input prefetch) is spread across the 16 steps of
the body; the output stage splits into a y-capture at body start and
the W_lin matmuls at body end.  A W@c+W@tmp' split-matmul variant was
tried and reverted: the doubled per-step PE instruction count cost
more on HW (~16ns/matmul of unmodeled issue overhead) than the
shortened chain saved.
"""

import sys

sys.path.insert(0, "/opt/trn_rl_repo")

import numpy as np

import concourse.bass as bass
import concourse.bacc as bacc_mod
import concourse.mybir as mybir
from concourse.bass import ds
from concourse.tile import TileContext

B, T_FULL, I, H, G = 64, 4096, 64, 256, 768
NCORES = 8
BL = B // NCORES  # 8 sequences per core
GRP = 16          # time steps per group
C = GRP * BL      # 128 columns per group (col = k*8 + b)
NJ = 6            # gate chunks of 128 (r: 0-1, z: 2-3, n: 4-5)
NI = 2            # hidden chunks of 128
NL = 2            # layers
F32 = mybir.dt.float32
F32R = mybir.dt.float32r
BF16 = mybir.dt.bfloat16
F8 = mybir.dt.bfloat16  # recurrent-weight storage dtype
WSCL = 1.0    # optional pre-activation scale folded into the weights
WINV = 1.0 / WSCL  # (undone at sigmoid scale= and in the n-path STT ops)

SIG = mybir.ActivationFunctionType.Sigmoid
TANH = mybir.ActivationFunctionType.Tanh
MULT = mybir.AluOpType.mult
ADD = mybir.AluOpType.add
SUB = mybir.AluOpType.subtract


def build_nc(T=T_FULL, unroll_all=False, sig_split=True, spread=True,
             debug=False):
    nc = bacc_mod.Bacc()

    NG = T // GRP
    RPAD_IN = (NG + 2) * C
    RPAD_OUT = (NG + 1) * C

    y_d = nc.declare_dram_parameter("y", [RPAD_IN, I], F32R, isOutput=False)
    z_d = nc.declare_dram_parameter("z", [RPAD_IN, I], F32, isOutput=False)
    whh0_d = nc.declare_dram_parameter("whh0", [128, NJ, NI, 128], F8,
                                       isOutput=False)
    whh1_d = nc.declare_dram_parameter("whh1", [128, NJ, NI, 128], F8,
                                       isOutput=False)
    wih1_d = nc.declare_dram_parameter("wih1", [128, NJ, NI, 128], BF16,
                                       isOutput=False)
    wih0a_d = nc.declare_dram_parameter("wih0a", [I + 1, NJ, 128], BF16,
                                        isOutput=False)
    wlin_d = nc.declare_dram_parameter("wlin", [128, NI, I], BF16,
                                       isOutput=False)
    bias1_d = nc.declare_dram_parameter("bias1", [1, NJ, 128], BF16,
                                        isOutput=False)
    blin_d = nc.declare_dram_parameter("blin", [1, I], BF16, isOutput=False)
    bcall_d = nc.declare_dram_parameter("bcall", [128, NL, NI, BL], BF16,
                                        isOutput=False)
    eyef_d = nc.declare_dram_parameter("eyef", [128, 128], F32,
                                       isOutput=False)
    eyeb_d = nc.declare_dram_parameter("eyeb", [128, 128], BF16,
                                       isOutput=False)
    eyer_d = nc.declare_dram_parameter("eyer", [128, 128], F32R,
                                       isOutput=False)
    out_d = nc.declare_dram_parameter("out", [RPAD_OUT, I], F32,
                                      isOutput=True)
    hdbg_d = None
    if debug:
        hdbg_d = nc.declare_dram_parameter(
            "hdbg", [NG + 1, 128, NL, NI, C], BF16, isOutput=True)

    with TileContext(nc) as tc:
        with (
            tc.tile_pool(name="wpool", bufs=1) as wpool,
            tc.tile_pool(name="gatep", bufs=6) as gatep,
            tc.tile_pool(name="iop", bufs=2) as iop,
            tc.tile_pool(name="ps_a", bufs=2, space="PSUM") as ps_a,
            tc.tile_pool(name="ps_n", bufs=2, space="PSUM") as ps_n,
            tc.tile_pool(name="ps_gx", bufs=2, space="PSUM") as ps_gx,
            tc.tile_pool(name="ps_tp", bufs=1, space="PSUM") as ps_tp,
            tc.tile_pool(name="ps_fin", bufs=1, space="PSUM") as ps_fin,
        ):
            # ---- persistent weights / constants ----
            whh0_t = wpool.tile([128, NJ, NI, 128], F8)
            whh1_t = wpool.tile([128, NJ, NI, 128], F8)
            wih1_t = wpool.tile([128, NJ, NI, 128], BF16)
            wih0a_t = wpool.tile([I + 1, NJ, 128], BF16)
            wlin_t = wpool.tile([128, NI, I], BF16)
            bias1_t = wpool.tile([1, NJ, 128], BF16)
            blin_t = wpool.tile([1, I], BF16)
            bcall_t = wpool.tile([128, NL, NI, BL], BF16)
            bczero_t = wpool.tile([128, NL, NI, BL], BF16)
            eyef_t = wpool.tile([128, 128], F32)
            eyeb_t = wpool.tile([128, 128], BF16)
            eyer_t = wpool.tile([128, 128], F32R)
            ones1_t = wpool.tile([1, 128], BF16)
            onesf_t = wpool.tile([1, 128], F32)

            nc.sync.dma_start(out=whh0_t, in_=whh0_d[:])
            nc.sync.dma_start(out=whh1_t, in_=whh1_d[:])
            nc.sync.dma_start(out=wih1_t, in_=wih1_d[:])
            nc.sync.dma_start(out=wih0a_t, in_=wih0a_d[:])
            nc.sync.dma_start(out=wlin_t, in_=wlin_d[:])
            nc.sync.dma_start(out=bias1_t, in_=bias1_d[:])
            nc.sync.dma_start(out=blin_t, in_=blin_d[:])
            nc.sync.dma_start(out=bcall_t, in_=bcall_d[:])
            nc.sync.dma_start(out=eyef_t, in_=eyef_d[:])
            nc.sync.dma_start(out=eyeb_t, in_=eyeb_d[:])
            nc.sync.dma_start(out=eyer_t, in_=eyer_d[:])
            nc.gpsimd.memset(onesf_t[:], 1.0)
            nc.vector.tensor_copy(ones1_t[:], onesf_t[:])
            # first-body N seed: real b_hh0_n for L0, zeros for L1 (keeps
            # the L1 state identically zero through the warm-up body)
            nc.vector.memset(bczero_t[:].bitcast(BF16), 0.0)
            nc.vector.tensor_copy(bczero_t[:, 0, :, :], bcall_t[:, 0, :, :])

            # ---- parity-pair state tiles ----
            # G_[p]: gx for (L0 group g, L1 group g-1), k-major
            G_ = [wpool.tile([128, GRP, NL, NJ, BL], BF16, name=f"G_{p}")
                  for p in range(2)]
            # H_[p]: [l, i, col]; l=0 -> h1 group g, l=1 -> h2 group g-1
            H_ = [wpool.tile([128, NL, NI, C], BF16, name=f"H_{p}")
                  for p in range(2)]
            # split-matmul carriers: c = z*h, tp = (1-z)*n, k-major
            c_ = [wpool.tile([128, GRP, NL, NI, BL], BF16, name=f"c_{p}")
                  for p in range(2)]
            tp_ = [wpool.tile([128, GRP, NL, NI, BL], BF16, name=f"tp_{p}")
                   for p in range(2)]
            xTa_ = [wpool.tile([I + 1, 128], BF16, name=f"xTa_{p}")
                    for p in range(2)]
            y_ = [wpool.tile([128, I], F32R, name=f"y_{p}") for p in range(2)]
            z_ = [wpool.tile([128, I], F32, name=f"z_{p}") for p in range(2)]

            # init
            nc.vector.tensor_copy(xTa_[0][I: I + 1, :], onesf_t[:])
            nc.vector.tensor_copy(xTa_[1][I: I + 1, :], onesf_t[:])
            nc.vector.memset(H_[1][:], 0.0)
            nc.vector.memset(c_[1][:], 0.0)
            nc.vector.memset(tp_[1][:], 0.0)
            nc.vector.memset(G_[0][:], 0.0)  # L0 slice overwritten below
            nc.vector.memset(y_[1][:].bitcast(F32), 0.0)

            def step(k, par, first):
                """One merged (L0+L1) GRU time step."""
                Gp, Hp = G_[par], H_[par]
                cp, tpp = c_[par], tp_[par]
                if k == 0:
                    csrc, tsrc = c_[1 - par], tp_[1 - par]
                    kprev = GRP - 1
                    Hhp = H_[1 - par]
                    colp = C - BL
                else:
                    csrc, tsrc = cp, tpp
                    kprev = k - 1
                    Hhp = Hp
                    colp = (k - 1) * BL
                A = ps_a.tile([128, NL, 4, BL], F32, tag="A")
                N = ps_n.tile([128, NL, 2, BL], F32, tag="N")
                # folds: seed A with gx(r,z) of both layers in ONE matmul
                # (two start=True matmuls into one bank would clear each
                # other's has_written state), N with b_hh_n
                nc.tensor.matmul(A[:, :, :, :], eyeb_t,
                                 Gp[:, k, :, 0:4, :],
                                 start=True, stop=False,
                                 skip_group_check=True)
                nc.tensor.matmul(N[:, :, :, :], eyeb_t,
                                 bczero_t[:] if first else bcall_t[:],
                                 start=True, stop=False,
                                 skip_group_check=True)
                # single W @ h block (r, z, n order: SIG_r's region
                # completes first, the n writes (m's input) stream last)
                for jgrp in ((0, 1), (2, 3), (4, 5)):
                    for l in range(NL):
                        W = whh0_t if l == 0 else whh1_t
                        for j in jgrp:
                            dst = (A[:, l, j, :] if j < 4
                                   else N[:, l, j - 4, :])
                            for i in range(NI):
                                stop = (l == NL - 1 and i == NI - 1
                                        and j in (3, NJ - 1))
                                nc.tensor.matmul(
                                    dst, W[:, j, i, :],
                                    Hhp[:, l, i, colp: colp + BL],
                                    start=False, stop=stop,
                                    skip_group_check=True)
                S = gatep.tile([128, NL, 4, BL], BF16, tag="S")
                if sig_split:
                    nc.scalar.activation(S[:, :, 0:2, :], A[:, :, 0:2, :],
                                         SIG, scale=WINV)
                    nc.scalar.activation(S[:, :, 2:4, :], A[:, :, 2:4, :],
                                         SIG, scale=WINV)
                else:
                    nc.scalar.activation(S, A, SIG, scale=WINV)
                # all chain+off ops on DVE/ACT only: buffer-reuse anti-deps
                # then fold into sem floors these engines already track, so
                # every instruction carries at most ONE fresh wait (parkable
                # in the engine wait queue instead of blocking the seq).
                m = gatep.tile([128, NL, 2, BL], F32, tag="m")
                nc.vector.scalar_tensor_tensor(m, N, WINV, S[:, :, 0:2, :],
                                               MULT, MULT)
                t = gatep.tile([128, NL, 2, BL], BF16, tag="t")
                nc.vector.scalar_tensor_tensor(t, Gp[:, k, :, 4:6, :], WINV,
                                               m, MULT, ADD)
                zm1 = gatep.tile([128, NL, 2, BL], BF16, tag="zm1")
                nc.vector.tensor_scalar(zm1, S[:, :, 2:4, :], -1.0, 1.0,
                                        MULT, ADD)
                nc.vector.tensor_tensor(
                    cp[:, k, :, :, :], S[:, :, 2:4, :],
                    Hhp[:, :, :, colp: colp + BL], MULT)
                nt = gatep.tile([128, NL, 2, BL], BF16, tag="nt")
                nc.scalar.activation(nt, t, TANH)
                nc.vector.tensor_tensor(tpp[:, k, :, :, :], nt, zm1, MULT)
                nc.vector.tensor_tensor(
                    Hp[:, :, :, k * BL: (k + 1) * BL],
                    cp[:, k, :, :, :], tpp[:, k, :, :, :], ADD)

            def gx_copy(j, dst, gps):
                if j % 2 == 0:
                    nc.vector.tensor_copy(dst, gps)
                else:
                    nc.scalar.copy(dst, gps)

            def gx1_piece(par, j, c0, c1):
                """W_ih1 @ H1(g) cols [c0,c1) for gate chunk j ->
                G_[1-par] L1 slice."""
                w = c1 - c0
                gps = ps_gx.tile([128, w], F32, tag="gx")
                for i in range(NI):
                    nc.tensor.matmul(gps, wih1_t[:, j, i, :],
                                     H_[par][:, 0, i, c0:c1],
                                     start=(i == 0), stop=False,
                                     skip_group_check=True)
                nc.tensor.matmul(gps, bias1_t[:, j, :], ones1_t[:, 0:w],
                                 start=False, stop=True,
                                 skip_group_check=True)
                k0, k1 = c0 // BL, c1 // BL
                gx_copy(j, G_[1 - par][:, k0:k1, 1, j, :], gps)

            def gx0_piece(par, j):
                """W_ih0a @ xTa(g+1) for gate chunk j -> G_[1-par] L0."""
                gps = ps_gx.tile([128, C], F32, tag="gx")
                nc.tensor.matmul(gps, wih0a_t[:, j, :], xTa_[1 - par],
                                 start=True, stop=True)
                gx_copy(j, G_[1 - par][:, :, 0, j, :], gps)

            def final_head(par):
                """Seed fp with y(g-1) + b_lin at body start, before the
                y buffer is overwritten by the next group's prefetch."""
                fp = ps_fin.tile([128, I], F32, tag="fin")
                nc.tensor.matmul(fp, eyer_t, y_[1 - par],
                                 start=True, stop=False)
                nc.tensor.matmul(fp, ones1_t, blin_t,
                                 start=False, stop=False)
                return fp

            def final_tail(fp, r_o, par):
                """fp += W_lin @ H2(g-1) (complete at body end); store."""
                nc.tensor.matmul(fp, H_[par][:, 1, 0, :], wlin_t[:, 0, :],
                                 start=False, stop=False)
                nc.tensor.matmul(fp, H_[par][:, 1, 1, :], wlin_t[:, 1, :],
                                 start=False, stop=True)
                o_t = iop.tile([128, I], F32, tag="o")
                nc.scalar.copy(o_t, fp)
                nc.sync.dma_start(out=out_d[ds(r_o, C), :], in_=o_t)

            def bulk0_head(r_y, par):
                nc.sync.dma_start(out=y_[par], in_=y_d[ds(r_y, C), :])
                nc.sync.dma_start(out=z_[par], in_=z_d[ds(r_y, C), :])

            def bulk0_mid(par):
                x_t = iop.tile([128, I], F32, tag="x")
                nc.gpsimd.tensor_tensor(x_t, y_[par].bitcast(F32), z_[par],
                                        ADD)
                tp = ps_tp.tile([I, 128], F32, tag="tp")
                nc.tensor.transpose(tp, x_t, eyef_t)
                nc.scalar.copy(xTa_[par][0:I, :], tp)

            def body(r0, par, first=False):
                fp = None
                for k in range(GRP):
                    step(k, par, first)
                    if not spread:
                        continue
                    if k == 0:
                        # must precede bulk0_head: reads y of group g-1
                        fp = final_head(par)
                    elif k == 1:
                        # prefetch next group's inputs into parity 1-par
                        bulk0_head(r0 + C, 1 - par)
                    elif k == 8:
                        bulk0_mid(1 - par)
                    elif 9 <= k <= 14:
                        gx0_piece(par, k - 9)
                        if k >= 10:
                            gx1_piece(par, k - 10, 0, 64)
                if not spread:
                    fp = final_head(par)
                    for j in range(NJ):
                        gx1_piece(par, j, 0, C)
                    final_tail(fp, r0, par)
                    bulk0_head(r0 + C, 1 - par)
                    bulk0_mid(1 - par)
                    for j in range(NJ):
                        gx0_piece(par, j)
                else:
                    gx1_piece(par, 5, 0, 64)
                    final_tail(fp, r0, par)
                    gx1_piece(par, 4, 64, C)
                    gx1_piece(par, 5, 64, C)
                    for j in range(4):
                        gx1_piece(par, j, 64, C)
                if debug:
                    nc.sync.dma_start(out=hdbg_d[r0 // C], in_=H_[par][:])

            # prologue: group 0 L0 inputs -> parity 0
            bulk0_head(0, 0)
            bulk0_mid(0)
            for j in range(NJ):
                gx0_piece(1, j)  # writes G_[0] L0 slice

            if unroll_all:
                for g in range(NG + 1):
                    body(g * C, g % 2, first=(g == 0))
            else:
                body(0, 0, first=True)
                with tc.For_i(C, (NG + 1) * C, 2 * C,
                              staggered_reset=True) as iv:
                    body(iv, 1)
                    body(iv + C, 0)

    nc.compile()
    return nc


def prep_weights(W_ih0, W_hh0, b_ih0, b_hh0, W_ih1, W_hh1, b_ih1, b_hh1,
                 W_lin, b_lin):
    """Host-side weight folding into gate-major bf16 layouts."""
    import ml_dtypes
    bf = ml_dtypes.bfloat16
    f8 = ml_dtypes.bfloat16
    f = np.float32
    s = f(WSCL)

    def _lay(W):  # [768, 256] -> [128, 6, 2, 128]
        return np.ascontiguousarray(
            W.reshape(NJ, 128, NI, 128).transpose(3, 0, 2, 1))

    # recurrent weights: fp8e3m4 at x WSCL; every other contribution to the
    # gate pre-activations (gx weights, biases) carries the same x WSCL so
    # the PSUM accumulation is uniformly scaled, undone at sigmoid (scale=)
    # and in the n-path STT ops.
    bias0 = (b_ih0 + np.concatenate([b_hh0[: 2 * H], np.zeros(H, f)])).astype(f)
    bias1 = (b_ih1 + np.concatenate([b_hh1[: 2 * H], np.zeros(H, f)])).astype(f)

    wih0a = np.zeros((I + 1, NJ, 128), f)
    wih0a[:I] = W_ih0.reshape(NJ, 128, I).transpose(2, 0, 1)
    wih0a[I] = bias0.reshape(NJ, 128)

    # bcall: [128, layer, i, BL] = b_hh_l n-part broadcast over batch
    bcall = np.zeros((128, NL, NI, BL), f)
    for l, bh in enumerate((b_hh0, b_hh1)):
        bcall[:, l] = np.broadcast_to(
            bh[2 * H:].reshape(NI, 128).T[:, :, None], (128, NI, BL))

    return {
        "whh0": (_lay(W_hh0) * s).astype(f8),
        "whh1": (_lay(W_hh1) * s).astype(f8),
        "wih1": (_lay(W_ih1) * s).astype(bf),
        "wih0a": (wih0a * s).astype(bf),
        "wlin": np.ascontiguousarray(
            W_lin.T.reshape(NI, 128, I).transpose(1, 0, 2)).astype(bf),
        "bias1": (bias1.reshape(1, NJ, 128) * s).astype(bf),
        "blin": b_lin.reshape(1, I).astype(bf),
        "bcall": np.ascontiguousarray(bcall * s).astype(bf),
        "eyef": np.eye(128, dtype=f),
        "eyeb": np.eye(128, dtype=bf),
        "eyer": np.eye(128, dtype=f),
    }


def prep_seq(a, T):
    """[BLc, T, I] f32 -> padded [RPAD_IN, I] rows (t*BLc+b order)."""
    BLc = a.shape[0]
    NG = T // GRP
    r = np.ascontiguousarray(a.transpose(1, 0, 2)).reshape(T * BLc, I)
    pad = np.zeros(((NG + 2) * GRP * BLc - T * BLc, I), np.float32)
    return np.concatenate([r, pad], axis=0)


def unprep_out(o, T):
    """[RPAD_OUT, I] -> [BL, T, I] (drop first pad group)."""
    o = o[C:].reshape(T, BL, I)
    return np.ascontiguousarray(o.transpose(1, 0, 2))


_NC_CACHE = {}


def kernel(z, y, W_ih0, W_hh0, b_ih0, b_hh0, W_ih1, W_hh1, b_ih1, b_hh1,
           W_lin, b_lin, _trace=False):
    """Full-input entry point: shards over 8 cores, returns full output."""
    from concourse.bass_utils import run_bass_kernel_spmd

    z = np.asarray(z, np.float32)
    y = np.asarray(y, np.float32)
    T = z.shape[1]
    if T not in _NC_CACHE:
        _NC_CACHE[T] = build_nc(T=T)
    nc = _NC_CACHE[T]

    wmaps = prep_weights(
        np.asarray(W_ih0), np.asarray(W_hh0), np.asarray(b_ih0),
        np.asarray(b_hh0), np.asarray(W_ih1), np.asarray(W_hh1),
        np.asarray(b_ih1), np.asarray(b_hh1), np.asarray(W_lin),
        np.asarray(b_lin))

    in_maps = []
    for cid in range(NCORES):
        sl = slice(cid * BL, (cid + 1) * BL)
        m = {"y": prep_seq(y[sl], T), "z": prep_seq(z[sl], T)}
        m.update(wmaps)
        in_maps.append(m)

    res = run_bass_kernel_spmd(nc, in_maps, list(range(NCORES)),
                               trace=_trace)
    outs = [unprep_out(res.results[cid]["out"], T) for cid in range(NCORES)]
    full = np.concatenate(outs, axis=0).astype(np.float32)
    if _trace:
        return full, res
    return full
